# revision 32
# baseline (speedup 1.0000x reference)
"""Self-contained Trainium2 Bass kernel for the 3-layer stacked GRU encoder
(nn_NoisyGRUSeq2SeqWithFeatures).

Strategy: 8-way MODEL-parallel (output-channel sharding) so the weights stay
SBUF-resident across the 64-step recurrence (6.8MB/core in fp16); the full
batch (B=128) is replicated on every core so every matmul runs with a full
128-wide stationary operand.  Per wave we run a layer-skewed schedule
(L0 at t, L1 at t-1, L2 at t-2) which lets the six per-step AllGathers
fuse into two (the structural minimum: the rh exchange causally depends on
the h exchange within a step).

Perf-relevant choices (measured via in-NEFF repeat-count slopes):
 - fp16 matmul operands + fp16 collective payloads: all matmuls run at
   1 cycle/row (fp32r pays 4x when the output free size < 256) and the
   AllGather payload halves.
 - the sequence mask rides as one extra rank-1 accumulation into each gate
   matmul (mask column x z-indicator row), so full waves need a single
   sigmoid over one [B, 896] PSUM tile instead of three biased ones.
 - the 8 post-AllGather unpack DMAs are one strided DMA (rearranged AP).
 - `repeat` re-runs the wave loop in-NEFF with h reset, so wall-clock
   slopes over repeat counts isolate device time from the ~150ms axon
   dispatch overhead.

Per-wave device time is ~88us: 2 AllGathers x ~22us (mostly fixed cost;
halving the payload saves only ~4us each) + a ~44us serial staging chain
whose latency is dependency hops, not engine throughput (sim: PE is only
50% busy; with staging deps cut the same matmuls pipeline at 13us/wave).
Remote-DMA SBUF-to-SBUF exchange would cut the collective cost ~5x but the
gpsimd `remote_dma` library ucode is absent from this bedrock image (hard
device crash when loaded).
"""

import numpy as np

SIZES = (512, 1024, 2048)
EMB = 32
VOC = 40
LATENT = 512
B, S = 128, 64
NCORES = 8

# per-core output slice sizes per layer
SL = tuple(sz // NCORES for sz in SIZES)  # (64, 128, 256)
HLOC = sum(SL)  # 448 columns of per-core h state
F32 = None  # set after mybir import
SKIP_COLL = False  # debug: skip collectives to isolate their cost
USE_FP16 = True  # fp16 matmul operands + fp16 collective payloads
SIM_VARIANT = "base"  # timing-only sim ablations (see sim_decomp.py)


def _npdt():
    return np.float16 if USE_FP16 else np.float32


def _sbufize(w: np.ndarray, tp: int = 128) -> np.ndarray:
    """[nk*tp, ncols] row-tiled weight -> SBUF layout [tp, nk*ncols]."""
    nk = w.shape[0] // tp
    assert w.shape[0] == nk * tp
    return (
        np.ascontiguousarray(w.reshape(nk, tp, w.shape[1]).transpose(1, 0, 2))
        .reshape(tp, nk * w.shape[1])
        .astype(_npdt())
    )


def prepack(inputs: dict) -> list[dict]:
    """Build per-core in_maps from the full (unsharded) problem inputs."""
    seqs = np.asarray(inputs["input_seqs"])
    lens = np.asarray(inputs["input_lens"])
    emb = np.asarray(inputs["emb"], np.float32)
    Kg = [np.asarray(inputs[f"Kg{l}"], np.float32) for l in range(3)]
    bg = [np.asarray(inputs[f"bg{l}"], np.float32) for l in range(3)]
    Kc = [np.asarray(inputs[f"Kc{l}"], np.float32) for l in range(3)]
    bc = [np.asarray(inputs[f"bc{l}"], np.float32) for l in range(3)]
    W_enc = np.asarray(inputs["W_enc"], np.float32)
    b_enc = np.asarray(inputs["b_enc"], np.float32)

    # shared tensors
    dt = _npdt()
    x_emb = emb[seqs]  # [B,S,EMB]
    xembT = np.zeros((EMB + 1, S * B), dt)
    for t in range(S):
        xembT[:EMB, t * B : (t + 1) * B] = x_emb[:, t, :].T
    xembT[EMB, :] = 1.0
    maskT = np.zeros((1, S * B), dt)
    for t in range(S):
        maskT[0, t * B : (t + 1) * B] = np.where(t < lens, 0.0, 30.0)
    ident = np.eye(128, dtype=np.float32)
    ones_row = np.ones((1, B), dt)

    xin = (EMB, SIZES[0], SIZES[1])  # x-input width per layer
    in_maps = []
    for i in range(NCORES):
        m = {
            "xembT": xembT,
            "maskT": maskT,
            "ident": ident,
            "ones_row": ones_row,
        }
        for l in range(3):
            zr = np.zeros((1, 2 * SL[l]), dt)
            zr[0, SL[l]:] = 1.0
            m[f"zind{l}"] = zr
        ccols = {}
        for l in range(3):
            sl, cout = SL[l], SIZES[l]
            rcols = sl * i + np.arange(sl)
            gcols = np.concatenate([rcols, cout + rcols])
            ccols[l] = rcols
            cin = xin[l]
            if l == 0:
                m["kg0x"] = np.vstack(
                    [Kg[0][:cin, gcols], bg[0][gcols][None, :]]
                ).astype(dt)
                m["kc0x"] = np.vstack(
                    [Kc[0][:cin, ccols[0]], bc[0][ccols[0]][None, :]]
                ).astype(dt)
            else:
                tpx = 64 if l == 1 else 128  # L1 x-input is h0 (64-row rank chunks)
                m[f"kg{l}x"] = _sbufize(Kg[l][:cin, gcols], tpx)
                m[f"kc{l}x"] = _sbufize(Kc[l][:cin, ccols[l]], tpx)
                m[f"bg{l}row"] = bg[l][gcols][None, :].astype(dt)
                m[f"bc{l}row"] = bc[l][ccols[l]][None, :].astype(dt)
            tp = 64 if l == 0 else 128
            m[f"kg{l}h"] = _sbufize(Kg[l][cin:, gcols], tp)
            m[f"kc{l}h"] = _sbufize(Kc[l][cin:, ccols[l]], tp)
        # W_enc rows for this core's h slices, in h_loc order, padded to 512
        wrows = np.concatenate(
            [
                512 + 128 * i + np.arange(128),
                1536 + 256 * i + np.arange(256),
                64 * i + np.arange(64),
            ]
        )
        wenc = np.zeros((512, LATENT), np.float32)
        wenc[:HLOC] = W_enc[wrows]
        m["wenc"] = _sbufize(wenc)
        m["benc_row"] = (b_enc / NCORES)[None, :].astype(dt)
        in_maps.append(m)
    return in_maps


def build(n_waves=S + 2, repeat=1):
    import concourse.bass as bass
    import concourse.bacc as bacc
    import concourse.tile as tile
    import concourse.mybir as mybir

    f32 = mybir.dt.float32
    AF = mybir.ActivationFunctionType
    nc = bacc.Bacc("TRN2", target_bir_lowering=False, debug=False, num_devices=NCORES)

    # h_loc column layout: [h1 (0:128) | h2 (128:384) | h0 (384:448)]
    # rank chunk (transposed, padded to 512 rows): same order + 64 pad rows
    dshapes = {
        "xembT": [EMB + 1, S * B],
        "maskT": [1, S * B],
        "zind0": [1, 2 * SL[0]],
        "zind1": [1, 2 * SL[1]],
        "zind2": [1, 2 * SL[2]],
        "ident": [128, 128],
        "ones_row": [1, B],
        "kg0x": [EMB + 1, 2 * SL[0]],
        "kc0x": [EMB + 1, SL[0]],
        "kg0h": [64, 8 * 2 * SL[0]],
        "kc0h": [64, 8 * SL[0]],
        "kg1x": [64, 8 * 2 * SL[1]],
        "kc1x": [64, 8 * SL[1]],
        "kg1h": [128, 8 * 2 * SL[1]],
        "kc1h": [128, 8 * SL[1]],
        "bg1row": [1, 2 * SL[1]],
        "bc1row": [1, SL[1]],
        "kg2x": [128, 8 * 2 * SL[2]],
        "kc2x": [128, 8 * SL[2]],
        "kg2h": [128, 16 * 2 * SL[2]],
        "kc2h": [128, 16 * SL[2]],
        "bg2row": [1, 2 * SL[2]],
        "bc2row": [1, SL[2]],
        "wenc": [128, 4 * LATENT],
        "benc_row": [1, LATENT],
    }
    f32r = mybir.dt.float32r
    dt16 = mybir.dt.float16 if USE_FP16 else f32r

    def ddt(k):
        if k == "ident":
            return f32r
        return dt16

    dram = {
        k: nc.dram_tensor(k, v, ddt(k), kind="ExternalInput")
        for k, v in dshapes.items()
    }
    out_d = nc.dram_tensor("out", [B, LATENT], f32, kind="ExternalOutput")

    with tile.TileContext(nc) as tc:
        with (
            tc.tile_pool(name="wts", bufs=1) as wp,
            tc.tile_pool(name="acts", bufs=1) as ap,
            tc.tile_pool(name="hbuf", bufs=1) as hp,
            tc.tile_pool(name="stg", bufs=2) as sp,
            tc.tile_pool(name="gates", bufs=1) as gp,
            tc.tile_pool(name="psg", bufs=1, space="PSUM") as psg,
            tc.tile_pool(name="psc", bufs=1, space="PSUM") as psc,
            tc.tile_pool(name="pst", bufs=2, space="PSUM") as pst,
            tc.tile_pool(name="dram", bufs=2, space="DRAM") as dp,
        ):
            w = {}
            for k in dshapes:
                t = wp.tile(dshapes[k], ddt(k), name=f"w_{k}")
                nc.sync.dma_start(t[:], dram[k][:])
                w[k] = t

            def wt(name, ncols, j, tp=128):
                return w[name][0:tp, j * ncols : (j + 1) * ncols]

            f32r = mybir.dt.float32r

            def mm(out, lhsT, rhs, **kw):
                nc.tensor.matmul(out, lhsT, rhs, **kw)

            simv = SIM_VARIANT
            RR = range(0, NCORES, 2) if simv == "halfmm" else range(NCORES)
            AA = (0,) if simv == "halfmm" else (0, 1)

            def emit(out, ops):
                if simv == "nomm":
                    ops = ops[:1]
                for i, (l_, r_) in enumerate(ops):
                    mm(out, l_, r_, start=(i == 0), stop=(i == len(ops) - 1))

            h_loc = ap.tile([B, HLOC], f32r, name="h_loc")
            rh_loc = ap.tile([B, HLOC], f32r, name="rh_loc")

            def fresh_hT(tag, uid):
                return hp.tile(
                    [128, NCORES * 4 * B], dt16, name=f"{tag}T{uid}", tag=f"{tag}T"
                )

            def RK(t, r, k):
                return t[:, (4 * r + k) * B : (4 * r + k + 1) * B]

            def RK64(t, r):
                return t[0:64, (4 * r + 3) * B : (4 * r + 3) * B + B]

            _stale = {}

            def stage_and_gather(src_loc, tag, wv):
                if simv == "nostage":
                    if tag not in _stale:
                        t = fresh_hT(tag, "stale")
                        nc.vector.memset(t[:].bitcast(f32), 0.0)
                        _stale[tag] = t
                    return _stale[tag]
                pt = pst.tile([128, 512], f32r, name=f"pt_{tag}{wv}", tag="pt")
                for k in range(4):
                    csz = 128 if k < 3 else HLOC - 384
                    nc.tensor.transpose(
                        pt[:csz, k * 128 : k * 128 + 128],
                        src_loc[:, k * 128 : k * 128 + csz],
                        w["ident"][:],
                    )
                stg = sp.tile([128, 512], dt16, name=f"stg_{tag}{wv}", tag=f"stg{tag}")
                nc.vector.tensor_copy(stg[:], pt[:])
                agin = dp.tile([128, 512], dt16, name=f"agin_{tag}{wv}", tag=f"agin{tag}")
                agout = dp.tile(
                    [NCORES * 128, 512], dt16,
                    name=f"agout_{tag}{wv}", tag=f"agout{tag}", addr_space="Shared",
                )
                nc.sync.dma_start(agin[:], stg[:])
                if SKIP_COLL:
                    nc.sync.dma_start(agout[0:128, :], agin[:])
                else:
                    nc.gpsimd.collective_compute(
                        "AllGather",
                        mybir.AluOpType.bypass,
                        replica_groups=[list(range(NCORES))],
                        ins=[agin[:]],
                        outs=[agout[:]],
                    )
                gT = fresh_hT(tag, wv)
                nc.sync.dma_start(
                    gT[:],
                    agout[:].rearrange("(r p) c -> p r c", r=NCORES),
                )
                return gT

            for gwv in range(repeat * n_waves):
                wv = gwv % n_waves
                if wv == 0:
                    nc.vector.memset(h_loc[:].bitcast(f32), 0.0)
                    nc.vector.memset(rh_loc[:].bitcast(f32), 0.0)
                    hT = fresh_hT("h", f"i{gwv}")
                    nc.vector.memset(hT[:].bitcast(f32), 0.0)
                t0, t1, t2 = wv, wv - 1, wv - 2

                # ---------------- gates ----------------
                pg = psg.tile([B, 2 * HLOC], f32, name=f"pg_{gwv}", tag="pg")
                if 0 <= t2 < S:
                    ops = [(w["ones_row"][:], w["bg2row"][:]),
                           (w["maskT"][:, t2 * B : (t2 + 1) * B], w["zind2"][:])]
                    ops += [(RK(hT, r, 0), wt("kg2x", 2 * SL[2], r)) for r in RR]
                    ops += [(RK(hT, r, 1 + a), wt("kg2h", 2 * SL[2], 2 * r + a))
                            for r in RR for a in AA]
                    emit(pg[:, 0:512], ops)
                if 0 <= t1 < S:
                    ops = [(w["ones_row"][:], w["bg1row"][:]),
                           (w["maskT"][:, t1 * B : (t1 + 1) * B], w["zind1"][:])]
                    ops += [(RK64(hT, r), wt("kg1x", 2 * SL[1], r, 64)) for r in RR]
                    ops += [(RK(hT, r, 0), wt("kg1h", 2 * SL[1], r)) for r in RR]
                    emit(pg[:, 512:768], ops)
                if t0 < S:
                    ops = [(w["xembT"][:, t0 * B : (t0 + 1) * B], w["kg0x"][:]),
                           (w["maskT"][:, t0 * B : (t0 + 1) * B], w["zind0"][:])]
                    ops += [(RK64(hT, r), wt("kg0h", 2 * SL[0], r, 64)) for r in RR]
                    emit(pg[:, 768:896], ops)

                # ---------------- sigmoid + r*h ----------------
                gsb = gp.tile([B, 2 * HLOC], f32r, name=f"gsb_{gwv}", tag="gsb")
                full = (0 <= t2) and (t0 < S) and simv != "noelem"
                if full:
                    nc.scalar.activation(gsb[:], pg[:], AF.Sigmoid)
                else:
                    if simv != "noelem" and 0 <= t2 < S:
                        nc.scalar.activation(gsb[:, 0:512], pg[:, 0:512], AF.Sigmoid)
                    if simv != "noelem" and 0 <= t1 < S:
                        nc.scalar.activation(gsb[:, 512:768], pg[:, 512:768], AF.Sigmoid)
                    if simv != "noelem" and t0 < S:
                        nc.scalar.activation(gsb[:, 768:896], pg[:, 768:896], AF.Sigmoid)
                if simv != "noelem" and 0 <= t2 < S:
                    nc.vector.tensor_mul(rh_loc[:, 128:384], gsb[:, 0 : SL[2]], h_loc[:, 128:384])
                if simv != "noelem" and 0 <= t1 < S:
                    nc.vector.tensor_mul(rh_loc[:, 0:128], gsb[:, 512 : 512 + SL[1]], h_loc[:, 0:128])
                if simv != "noelem" and t0 < S:
                    nc.vector.tensor_mul(rh_loc[:, 384:HLOC], gsb[:, 768 : 768 + SL[0]], h_loc[:, 384:HLOC])

                # ---------------- AG(rh) ----------------
                rhT = stage_and_gather(rh_loc, "r", gwv)

                # ---------------- candidates ----------------
                pc = psc.tile([B, HLOC], f32, name=f"pc_{gwv}", tag="pc")
                if 0 <= t1 < S:
                    ops = [(w["ones_row"][:], w["bc1row"][:])]
                    ops += [(RK64(hT, r), wt("kc1x", SL[1], r, 64)) for r in RR]
                    ops += [(RK(rhT, r, 0), wt("kc1h", SL[1], r)) for r in RR]
                    emit(pc[:, 0:128], ops)
                if 0 <= t2 < S:
                    ops = [(w["ones_row"][:], w["bc2row"][:])]
                    ops += [(RK(hT, r, 0), wt("kc2x", SL[2], r)) for r in RR]
                    ops += [(RK(rhT, r, 1 + a), wt("kc2h", SL[2], 2 * r + a))
                            for r in RR for a in AA]
                    emit(pc[:, 128:384], ops)
                if t0 < S:
                    ops = [(w["xembT"][:, t0 * B : (t0 + 1) * B], w["kc0x"][:])]
                    ops += [(RK64(rhT, r), wt("kc0h", SL[0], r, 64)) for r in RR]
                    emit(pc[:, 384:HLOC], ops)

                # ---------------- tanh + h_new ----------------
                csb = gp.tile([B, HLOC], f32r, name=f"csb_{gwv}", tag="csb")
                if simv != "noelem":
                    nc.scalar.activation(csb[:], pc[:], AF.Tanh)
                tmp2 = gp.tile([B, SL[2]], f32r, name=f"tmp2_{gwv}", tag="tmp2")
                zsl = {
                    0: (gsb, 768 + SL[0], 384, HLOC),
                    1: (gsb, 512 + SL[1], 0, 128),
                    2: (gsb, SL[2], 128, 384),
                }
                for l, tl in ((0, t0), (1, t1), (2, t2)):
                    if tl < 0 or tl >= S or simv == "noelem":
                        continue
                    gt, zoff, a, b2 = zsl[l]
                    sw = b2 - a
                    nc.vector.tensor_sub(tmp2[:, :sw], h_loc[:, a:b2], csb[:, a:b2])
                    nc.vector.tensor_mul(tmp2[:, :sw], gt[:, zoff : zoff + sw], tmp2[:, :sw])
                    nc.vector.tensor_add(h_loc[:, a:b2], tmp2[:, :sw], csb[:, a:b2])

                # ---------------- AG(h) ----------------
                hT = stage_and_gather(h_loc, "h", gwv)

            # ---------------- final projection ----------------
            ptf = pst.tile([128, 512], f32r, name="ptf", tag="pt")
            for k in range(4):
                csz = 128 if k < 3 else HLOC - 384
                nc.tensor.transpose(
                    ptf[:csz, k * 128 : k * 128 + 128],
                    h_loc[:, k * 128 : k * 128 + csz],
                    w["ident"][:],
                )
            hsf = sp.tile([128, 512], dt16, name="hsf", tag="stgh")
            nc.vector.tensor_copy(hsf[:], ptf[:])
            nc.vector.memset(hsf[64:128, 384:512].bitcast(f32), 0.0)
            pz = psg.tile([B, LATENT], f32, name="pz", tag="pg")
            mm(pz[:], w["ones_row"][:], w["benc_row"][:], start=True, stop=False)
            for k in range(4):
                mm(pz[:], hsf[:, k * 128 : (k + 1) * 128],
                                 wt("wenc", LATENT, k), start=False, stop=(k == 3))
            zsb = gp.tile([B, LATENT], f32, name="zsb", tag="gsb")
            nc.vector.tensor_copy(zsb[:], pz[:])
            arin = dp.tile([B, LATENT], f32, name="arin")
            arout = dp.tile([B, LATENT], f32, name="arout", addr_space="Shared")
            nc.sync.dma_start(arin[:], zsb[:])
            if SKIP_COLL:
                nc.sync.dma_start(arout[:], arin[:])
            else:
                nc.gpsimd.collective_compute(
                    "AllReduce",
                    mybir.AluOpType.add,
                    replica_groups=[list(range(NCORES))],
                    ins=[arin[:]],
                    outs=[arout[:]],
                )
            zfull = gp.tile([B, LATENT], f32, name="zfull", tag="csb")
            nc.sync.dma_start(zfull[:], arout[:])
            ofin = gp.tile([B, LATENT], f32, name="ofin", tag="gsb")
            nc.scalar.activation(ofin[:], zfull[:], AF.Tanh)
            nc.sync.dma_start(out_d[:], ofin[:])

    nc.compile()
    return nc


_NC_CACHE = {}


def kernel(**inputs) -> np.ndarray:
    from concourse import bass_utils

    if "nc" not in _NC_CACHE:
        _NC_CACHE["nc"] = build()
    nc = _NC_CACHE["nc"]
    in_maps = prepack(inputs)
    res = bass_utils.run_bass_kernel_spmd(nc, in_maps, core_ids=list(range(NCORES)))
    return np.asarray(res.results[0]["out"], np.float32)



# revision 36
# speedup vs baseline: 1.1941x; 1.1941x over previous
"""Self-contained Trainium2 Bass kernel for the 3-layer stacked GRU encoder
(nn_NoisyGRUSeq2SeqWithFeatures).

Strategy: 8-way MODEL-parallel (output-channel sharding) so the weights stay
SBUF-resident across the 64-step recurrence (6.8MB/core in fp16); the full
batch (B=128) is replicated on every core so every matmul runs with a full
128-wide stationary operand.  Per wave we run a layer-skewed schedule
(L0 at t, L1 at t-1, L2 at t-2) which lets the six per-step AllGathers
fuse into two (the structural minimum: the rh exchange causally depends on
the h exchange within a step).

Perf-relevant choices (measured via in-NEFF repeat-count slopes):
 - fp16 matmul operands + fp16 collective payloads: all matmuls run at
   1 cycle/row (fp32r pays 4x when the output free size < 256) and the
   AllGather payload halves.
 - the sequence mask rides as one extra rank-1 accumulation into each gate
   matmul (mask column x z-indicator row), so full waves need a single
   sigmoid over one [B, 896] PSUM tile instead of three biased ones.
 - the 8 post-AllGather unpack DMAs are one strided DMA (rearranged AP).
 - `repeat` re-runs the wave loop in-NEFF with h reset, so wall-clock
   slopes over repeat counts isolate device time from the ~150ms axon
   dispatch overhead.

Per-wave device time is ~86-90us: 2 AllGathers x ~22us (mostly fixed cost;
halving the payload saves only ~4us each) + a ~44us serial staging chain
whose latency is dependency hops, not engine throughput (sim: PE is only
50% busy; with staging deps cut the same matmuls pipeline at 13us/wave).
Tried and rejected: PE-warming filler matmuls in the AG windows (DVFS
counter-measure) measured +9us/wave worse — the in-order PE queue drains
them before the post-AG matmuls.  Remote-DMA SBUF-to-SBUF exchange would
cut the collective cost ~5x but the gpsimd `remote_dma` library ucode is
absent from this bedrock image (hard device crash when loaded).
"""

import numpy as np

SIZES = (512, 1024, 2048)
EMB = 32
VOC = 40
LATENT = 512
B, S = 128, 64
NCORES = 8

# per-core output slice sizes per layer
SL = tuple(sz // NCORES for sz in SIZES)  # (64, 128, 256)
HLOC = sum(SL)  # 448 columns of per-core h state
F32 = None  # set after mybir import
SKIP_COLL = False  # debug: skip collectives to isolate their cost
USE_FP16 = True  # fp16 matmul operands + fp16 collective payloads
SIM_VARIANT = "base"  # timing-only sim ablations (see sim_decomp.py)
N_FILL = 0  # PE-warming fillers per AG window (measured: they hurt; keep 0)


def _npdt():
    return np.float16 if USE_FP16 else np.float32


def _sbufize(w: np.ndarray, tp: int = 128) -> np.ndarray:
    """[nk*tp, ncols] row-tiled weight -> SBUF layout [tp, nk*ncols]."""
    nk = w.shape[0] // tp
    assert w.shape[0] == nk * tp
    return (
        np.ascontiguousarray(w.reshape(nk, tp, w.shape[1]).transpose(1, 0, 2))
        .reshape(tp, nk * w.shape[1])
        .astype(_npdt())
    )


def prepack(inputs: dict) -> list[dict]:
    """Build per-core in_maps from the full (unsharded) problem inputs."""
    seqs = np.asarray(inputs["input_seqs"])
    lens = np.asarray(inputs["input_lens"])
    emb = np.asarray(inputs["emb"], np.float32)
    Kg = [np.asarray(inputs[f"Kg{l}"], np.float32) for l in range(3)]
    bg = [np.asarray(inputs[f"bg{l}"], np.float32) for l in range(3)]
    Kc = [np.asarray(inputs[f"Kc{l}"], np.float32) for l in range(3)]
    bc = [np.asarray(inputs[f"bc{l}"], np.float32) for l in range(3)]
    W_enc = np.asarray(inputs["W_enc"], np.float32)
    b_enc = np.asarray(inputs["b_enc"], np.float32)

    # shared tensors
    dt = _npdt()
    x_emb = emb[seqs]  # [B,S,EMB]
    xembT = np.zeros((EMB + 1, S * B), dt)
    for t in range(S):
        xembT[:EMB, t * B : (t + 1) * B] = x_emb[:, t, :].T
    xembT[EMB, :] = 1.0
    maskT = np.zeros((1, S * B), dt)
    for t in range(S):
        maskT[0, t * B : (t + 1) * B] = np.where(t < lens, 0.0, 30.0)
    ident = np.eye(128, dtype=np.float32)
    ones_row = np.ones((1, B), dt)

    xin = (EMB, SIZES[0], SIZES[1])  # x-input width per layer
    in_maps = []
    for i in range(NCORES):
        m = {
            "xembT": xembT,
            "maskT": maskT,
            "ident": ident,
            "identf": ident,
            "fillw": np.zeros((128, 512), np.float32),
            "ones_row": ones_row,
        }
        for l in range(3):
            zr = np.zeros((1, 2 * SL[l]), dt)
            zr[0, SL[l]:] = 1.0
            m[f"zind{l}"] = zr
        ccols = {}
        for l in range(3):
            sl, cout = SL[l], SIZES[l]
            rcols = sl * i + np.arange(sl)
            gcols = np.concatenate([rcols, cout + rcols])
            ccols[l] = rcols
            cin = xin[l]
            if l == 0:
                m["kg0x"] = np.vstack(
                    [Kg[0][:cin, gcols], bg[0][gcols][None, :]]
                ).astype(dt)
                m["kc0x"] = np.vstack(
                    [Kc[0][:cin, ccols[0]], bc[0][ccols[0]][None, :]]
                ).astype(dt)
            else:
                tpx = 64 if l == 1 else 128  # L1 x-input is h0 (64-row rank chunks)
                m[f"kg{l}x"] = _sbufize(Kg[l][:cin, gcols], tpx)
                m[f"kc{l}x"] = _sbufize(Kc[l][:cin, ccols[l]], tpx)
                m[f"bg{l}row"] = bg[l][gcols][None, :].astype(dt)
                m[f"bc{l}row"] = bc[l][ccols[l]][None, :].astype(dt)
            tp = 64 if l == 0 else 128
            m[f"kg{l}h"] = _sbufize(Kg[l][cin:, gcols], tp)
            m[f"kc{l}h"] = _sbufize(Kc[l][cin:, ccols[l]], tp)
        # W_enc rows for this core's h slices, in h_loc order, padded to 512
        wrows = np.concatenate(
            [
                512 + 128 * i + np.arange(128),
                1536 + 256 * i + np.arange(256),
                64 * i + np.arange(64),
            ]
        )
        wenc = np.zeros((512, LATENT), np.float32)
        wenc[:HLOC] = W_enc[wrows]
        m["wenc"] = _sbufize(wenc)
        m["benc_row"] = (b_enc / NCORES)[None, :].astype(dt)
        in_maps.append(m)
    return in_maps


def build(n_waves=S + 2, repeat=1):
    import concourse.bass as bass
    import concourse.bacc as bacc
    import concourse.tile as tile
    import concourse.mybir as mybir

    f32 = mybir.dt.float32
    AF = mybir.ActivationFunctionType
    nc = bacc.Bacc("TRN2", target_bir_lowering=False, debug=False, num_devices=NCORES)

    # h_loc column layout: [h1 (0:128) | h2 (128:384) | h0 (384:448)]
    # rank chunk (transposed, padded to 512 rows): same order + 64 pad rows
    dshapes = {
        "xembT": [EMB + 1, S * B],
        "maskT": [1, S * B],
        "zind0": [1, 2 * SL[0]],
        "zind1": [1, 2 * SL[1]],
        "zind2": [1, 2 * SL[2]],
        "ident": [128, 128],
        "identf": [128, 128],
        "fillw": [128, 512],
        "ones_row": [1, B],
        "kg0x": [EMB + 1, 2 * SL[0]],
        "kc0x": [EMB + 1, SL[0]],
        "kg0h": [64, 8 * 2 * SL[0]],
        "kc0h": [64, 8 * SL[0]],
        "kg1x": [64, 8 * 2 * SL[1]],
        "kc1x": [64, 8 * SL[1]],
        "kg1h": [128, 8 * 2 * SL[1]],
        "kc1h": [128, 8 * SL[1]],
        "bg1row": [1, 2 * SL[1]],
        "bc1row": [1, SL[1]],
        "kg2x": [128, 8 * 2 * SL[2]],
        "kc2x": [128, 8 * SL[2]],
        "kg2h": [128, 16 * 2 * SL[2]],
        "kc2h": [128, 16 * SL[2]],
        "bg2row": [1, 2 * SL[2]],
        "bc2row": [1, SL[2]],
        "wenc": [128, 4 * LATENT],
        "benc_row": [1, LATENT],
    }
    f32r = mybir.dt.float32r
    dt16 = mybir.dt.float16 if USE_FP16 else f32r

    def ddt(k):
        if k == "ident":
            return f32r
        if k in ("identf", "fillw"):
            return f32
        return dt16

    dram = {
        k: nc.dram_tensor(k, v, ddt(k), kind="ExternalInput")
        for k, v in dshapes.items()
    }
    out_d = nc.dram_tensor("out", [B, LATENT], f32, kind="ExternalOutput")

    with tile.TileContext(nc) as tc:
        with (
            tc.tile_pool(name="wts", bufs=1) as wp,
            tc.tile_pool(name="acts", bufs=1) as ap,
            tc.tile_pool(name="hbuf", bufs=1) as hp,
            tc.tile_pool(name="stg", bufs=2) as sp,
            tc.tile_pool(name="gates", bufs=1) as gp,
            tc.tile_pool(name="psg", bufs=1, space="PSUM") as psg,
            tc.tile_pool(name="psc", bufs=1, space="PSUM") as psc,
            tc.tile_pool(name="pst", bufs=2, space="PSUM") as pst,
            tc.tile_pool(name="psf", bufs=1, space="PSUM") as psf,
            tc.tile_pool(name="dram", bufs=2, space="DRAM") as dp,
        ):
            w = {}
            for k in dshapes:
                t = wp.tile(dshapes[k], ddt(k), name=f"w_{k}")
                nc.sync.dma_start(t[:], dram[k][:])
                w[k] = t

            def wt(name, ncols, j, tp=128):
                return w[name][0:tp, j * ncols : (j + 1) * ncols]

            f32r = mybir.dt.float32r

            def mm(out, lhsT, rhs, **kw):
                nc.tensor.matmul(out, lhsT, rhs, **kw)

            simv = SIM_VARIANT
            RR = range(0, NCORES, 2) if simv == "halfmm" else range(NCORES)
            AA = (0,) if simv == "halfmm" else (0, 1)

            def emit(out, ops):
                if simv == "nomm":
                    ops = ops[:1]
                for i, (l_, r_) in enumerate(ops):
                    mm(out, l_, r_, start=(i == 0), stop=(i == len(ops) - 1))

            def pe_warm(uid):
                # slow fp32 matmuls into a scratch psum: keep the PE clocked
                # up through an AllGather window (results never read)
                if N_FILL <= 0:
                    return
                pf = psf.tile([B, 512], f32, name=f"pf{uid}", tag="pf")
                for i in range(N_FILL):
                    mm(pf[:], w["identf"][:], w["fillw"][:], start=True, stop=True)

            h_loc = ap.tile([B, HLOC], f32r, name="h_loc")
            rh_loc = ap.tile([B, HLOC], f32r, name="rh_loc")

            def fresh_hT(tag, uid):
                return hp.tile(
                    [128, NCORES * 4 * B], dt16, name=f"{tag}T{uid}", tag=f"{tag}T"
                )

            def RK(t, r, k):
                return t[:, (4 * r + k) * B : (4 * r + k + 1) * B]

            def RK64(t, r):
                return t[0:64, (4 * r + 3) * B : (4 * r + 3) * B + B]

            _stale = {}

            def stage_and_gather(src_loc, tag, wv):
                if simv == "nostage":
                    if tag not in _stale:
                        t = fresh_hT(tag, "stale")
                        nc.vector.memset(t[:].bitcast(f32), 0.0)
                        _stale[tag] = t
                    return _stale[tag]
                pt = pst.tile([128, 512], f32r, name=f"pt_{tag}{wv}", tag="pt")
                for k in range(4):
                    csz = 128 if k < 3 else HLOC - 384
                    nc.tensor.transpose(
                        pt[:csz, k * 128 : k * 128 + 128],
                        src_loc[:, k * 128 : k * 128 + csz],
                        w["ident"][:],
                    )
                stg = sp.tile([128, 512], dt16, name=f"stg_{tag}{wv}", tag=f"stg{tag}")
                nc.vector.tensor_copy(stg[:], pt[:])
                agin = dp.tile([128, 512], dt16, name=f"agin_{tag}{wv}", tag=f"agin{tag}")
                agout = dp.tile(
                    [NCORES * 128, 512], dt16,
                    name=f"agout_{tag}{wv}", tag=f"agout{tag}", addr_space="Shared",
                )
                nc.sync.dma_start(agin[:], stg[:])
                if SKIP_COLL:
                    nc.sync.dma_start(agout[0:128, :], agin[:])
                else:
                    nc.gpsimd.collective_compute(
                        "AllGather",
                        mybir.AluOpType.bypass,
                        replica_groups=[list(range(NCORES))],
                        ins=[agin[:]],
                        outs=[agout[:]],
                    )
                gT = fresh_hT(tag, wv)
                nc.sync.dma_start(
                    gT[:],
                    agout[:].rearrange("(r p) c -> p r c", r=NCORES),
                )
                return gT

            for gwv in range(repeat * n_waves):
                wv = gwv % n_waves
                if wv == 0:
                    nc.vector.memset(h_loc[:].bitcast(f32), 0.0)
                    nc.vector.memset(rh_loc[:].bitcast(f32), 0.0)
                    hT = fresh_hT("h", f"i{gwv}")
                    nc.vector.memset(hT[:].bitcast(f32), 0.0)
                t0, t1, t2 = wv, wv - 1, wv - 2

                # ---------------- gates ----------------
                pg = psg.tile([B, 2 * HLOC], f32, name=f"pg_{gwv}", tag="pg")
                if 0 <= t2 < S:
                    ops = [(w["ones_row"][:], w["bg2row"][:]),
                           (w["maskT"][:, t2 * B : (t2 + 1) * B], w["zind2"][:])]
                    ops += [(RK(hT, r, 0), wt("kg2x", 2 * SL[2], r)) for r in RR]
                    ops += [(RK(hT, r, 1 + a), wt("kg2h", 2 * SL[2], 2 * r + a))
                            for r in RR for a in AA]
                    emit(pg[:, 0:512], ops)
                if 0 <= t1 < S:
                    ops = [(w["ones_row"][:], w["bg1row"][:]),
                           (w["maskT"][:, t1 * B : (t1 + 1) * B], w["zind1"][:])]
                    ops += [(RK64(hT, r), wt("kg1x", 2 * SL[1], r, 64)) for r in RR]
                    ops += [(RK(hT, r, 0), wt("kg1h", 2 * SL[1], r)) for r in RR]
                    emit(pg[:, 512:768], ops)
                if t0 < S:
                    ops = [(w["xembT"][:, t0 * B : (t0 + 1) * B], w["kg0x"][:]),
                           (w["maskT"][:, t0 * B : (t0 + 1) * B], w["zind0"][:])]
                    ops += [(RK64(hT, r), wt("kg0h", 2 * SL[0], r, 64)) for r in RR]
                    emit(pg[:, 768:896], ops)

                # ---------------- sigmoid + r*h ----------------
                gsb = gp.tile([B, 2 * HLOC], f32r, name=f"gsb_{gwv}", tag="gsb")
                full = (0 <= t2) and (t0 < S) and simv != "noelem"
                if full:
                    nc.scalar.activation(gsb[:], pg[:], AF.Sigmoid)
                else:
                    if simv != "noelem" and 0 <= t2 < S:
                        nc.scalar.activation(gsb[:, 0:512], pg[:, 0:512], AF.Sigmoid)
                    if simv != "noelem" and 0 <= t1 < S:
                        nc.scalar.activation(gsb[:, 512:768], pg[:, 512:768], AF.Sigmoid)
                    if simv != "noelem" and t0 < S:
                        nc.scalar.activation(gsb[:, 768:896], pg[:, 768:896], AF.Sigmoid)
                if simv != "noelem" and 0 <= t2 < S:
                    nc.vector.tensor_mul(rh_loc[:, 128:384], gsb[:, 0 : SL[2]], h_loc[:, 128:384])
                if simv != "noelem" and 0 <= t1 < S:
                    nc.vector.tensor_mul(rh_loc[:, 0:128], gsb[:, 512 : 512 + SL[1]], h_loc[:, 0:128])
                if simv != "noelem" and t0 < S:
                    nc.vector.tensor_mul(rh_loc[:, 384:HLOC], gsb[:, 768 : 768 + SL[0]], h_loc[:, 384:HLOC])

                # ---------------- AG(rh) ----------------
                rhT = stage_and_gather(rh_loc, "r", gwv)
                pe_warm(f"r{gwv}")

                # ---------------- candidates ----------------
                pc = psc.tile([B, HLOC], f32, name=f"pc_{gwv}", tag="pc")
                if 0 <= t1 < S:
                    ops = [(w["ones_row"][:], w["bc1row"][:])]
                    ops += [(RK64(hT, r), wt("kc1x", SL[1], r, 64)) for r in RR]
                    ops += [(RK(rhT, r, 0), wt("kc1h", SL[1], r)) for r in RR]
                    emit(pc[:, 0:128], ops)
                if 0 <= t2 < S:
                    ops = [(w["ones_row"][:], w["bc2row"][:])]
                    ops += [(RK(hT, r, 0), wt("kc2x", SL[2], r)) for r in RR]
                    ops += [(RK(rhT, r, 1 + a), wt("kc2h", SL[2], 2 * r + a))
                            for r in RR for a in AA]
                    emit(pc[:, 128:384], ops)
                if t0 < S:
                    ops = [(w["xembT"][:, t0 * B : (t0 + 1) * B], w["kc0x"][:])]
                    ops += [(RK64(rhT, r), wt("kc0h", SL[0], r, 64)) for r in RR]
                    emit(pc[:, 384:HLOC], ops)

                # ---------------- tanh + h_new ----------------
                csb = gp.tile([B, HLOC], f32r, name=f"csb_{gwv}", tag="csb")
                if simv != "noelem":
                    nc.scalar.activation(csb[:], pc[:], AF.Tanh)
                tmp2 = gp.tile([B, SL[2]], f32r, name=f"tmp2_{gwv}", tag="tmp2")
                zsl = {
                    0: (gsb, 768 + SL[0], 384, HLOC),
                    1: (gsb, 512 + SL[1], 0, 128),
                    2: (gsb, SL[2], 128, 384),
                }
                for l, tl in ((0, t0), (1, t1), (2, t2)):
                    if tl < 0 or tl >= S or simv == "noelem":
                        continue
                    gt, zoff, a, b2 = zsl[l]
                    sw = b2 - a
                    nc.vector.tensor_sub(tmp2[:, :sw], h_loc[:, a:b2], csb[:, a:b2])
                    nc.vector.tensor_mul(tmp2[:, :sw], gt[:, zoff : zoff + sw], tmp2[:, :sw])
                    nc.vector.tensor_add(h_loc[:, a:b2], tmp2[:, :sw], csb[:, a:b2])

                # ---------------- AG(h) ----------------
                hT = stage_and_gather(h_loc, "h", gwv)
                pe_warm(f"h{gwv}")

            # ---------------- final projection ----------------
            ptf = pst.tile([128, 512], f32r, name="ptf", tag="pt")
            for k in range(4):
                csz = 128 if k < 3 else HLOC - 384
                nc.tensor.transpose(
                    ptf[:csz, k * 128 : k * 128 + 128],
                    h_loc[:, k * 128 : k * 128 + csz],
                    w["ident"][:],
                )
            hsf = sp.tile([128, 512], dt16, name="hsf", tag="stgh")
            nc.vector.tensor_copy(hsf[:], ptf[:])
            nc.vector.memset(hsf[64:128, 384:512].bitcast(f32), 0.0)
            pz = psg.tile([B, LATENT], f32, name="pz", tag="pg")
            mm(pz[:], w["ones_row"][:], w["benc_row"][:], start=True, stop=False)
            for k in range(4):
                mm(pz[:], hsf[:, k * 128 : (k + 1) * 128],
                                 wt("wenc", LATENT, k), start=False, stop=(k == 3))
            zsb = gp.tile([B, LATENT], f32, name="zsb", tag="gsb")
            nc.vector.tensor_copy(zsb[:], pz[:])
            arin = dp.tile([B, LATENT], f32, name="arin")
            arout = dp.tile([B, LATENT], f32, name="arout", addr_space="Shared")
            nc.sync.dma_start(arin[:], zsb[:])
            if SKIP_COLL:
                nc.sync.dma_start(arout[:], arin[:])
            else:
                nc.gpsimd.collective_compute(
                    "AllReduce",
                    mybir.AluOpType.add,
                    replica_groups=[list(range(NCORES))],
                    ins=[arin[:]],
                    outs=[arout[:]],
                )
            zfull = gp.tile([B, LATENT], f32, name="zfull", tag="csb")
            nc.sync.dma_start(zfull[:], arout[:])
            ofin = gp.tile([B, LATENT], f32, name="ofin", tag="gsb")
            nc.scalar.activation(ofin[:], zfull[:], AF.Tanh)
            nc.sync.dma_start(out_d[:], ofin[:])

    nc.compile()
    return nc


_NC_CACHE = {}


def kernel(**inputs) -> np.ndarray:
    from concourse import bass_utils

    if "nc" not in _NC_CACHE:
        _NC_CACHE["nc"] = build()
    nc = _NC_CACHE["nc"]
    in_maps = prepack(inputs)
    res = bass_utils.run_bass_kernel_spmd(nc, in_maps, core_ids=list(range(NCORES)))
    return np.asarray(res.results[0]["out"], np.float32)

